# revision 16
# baseline (speedup 1.0000x reference)
"""Biaffine edge attention on 8 Trainium2 NeuronCores (bf16, host relayout).

out[b,i,j] = head[b,i,:] @ edge_U @ dep[b,j,:] + head[b,i,:]@w1 + dep[b,j,:]@w2 + b0

Sharding: data-parallel over batch (B=8, one batch per core). Device does the
two big GEMMs in bf16 (full PE column rate, fp32 PSUM accumulation):
  T1T[k,i] = sum_d U[d,k] * HT[d,i]          (mm1)
  out[i,j] = sum_k T1T[k,i] * PT[k,j] + s_head[i] + s_dep[j] + b0   (mm2 + STT)

Host prep (extends the previous U relayout): HT = head[b].T and PT = dep[b].T
are laid out so every DMA is 128 partitions x large contiguous chunks; the
rank-1 terms s_head = head@w1 and s_dep = dep@w2 + b0 (0.4% of FLOPs) are
computed on host and enter the epilogue as per-partition scalar + broadcast
row. This removes all PE transposes and the sdep matmuls from the PE stream,
leaving exactly the 256 unavoidable 512-column matmuls.

Warmup matmuls on a zeroed tile run while the first DMAs land so the HAM
clock ramp (0.86 -> 1.27 -> 2.4 GHz, activity-driven) completes before the
real stream. All loads ride the sync HWDGE ring in consumption order (the
ACT ring takes ~5us to start transferring); the first U column-pair + HT
slab dispatches are small so mm1 can start after ~0.5MB instead of 6MB --
with 8 cores loading at once the per-core HBM share is only ~200GB/s.
"""

import numpy as np
import ml_dtypes

import concourse.bass as bass  # noqa: F401  (side-effect: mybir registration)
import concourse.mybir as mybir
import concourse.tile as tile
from concourse import bacc
from concourse.bass_utils import run_bass_kernel_spmd

B, S, D = 8, 1024, 1024
P = 128
SO = S // P  # 8
DO = D // P  # 8
NH = 512     # matmul free-dim tile (one fp32 PSUM bank)
F32 = mybir.dt.float32
BF16 = mybir.dt.bfloat16
ADD = mybir.AluOpType.add
BF16NP = ml_dtypes.bfloat16

N_WARMUP = 10

_CACHE = {}


def build_nc():
    nc = bacc.Bacc(None, target_bir_lowering=False)

    # Slab-major host layouts; each dispatch reads one dense block:
    # ht[s, dd, j, i] = headT[(2s+j)*P + dd, i].
    ht_d = nc.dram_tensor("ht", [DO // 2, P, 2, S], BF16, kind="ExternalInput")
    pt_d = nc.dram_tensor("pt", [2, P, DO // 2, S], BF16, kind="ExternalInput")
    # u[s, dd, kt', do, k] = U[do*P+dd, (2s+kt')*P+k]
    u_d = nc.dram_tensor("u", [DO // 2, P, 2, DO, P], BF16,
                         kind="ExternalInput")
    shead_d = nc.dram_tensor("shead", [P, SO], F32, kind="ExternalInput")
    sdep_d = nc.dram_tensor("sdep", [1, S], F32, kind="ExternalInput")
    out_d = nc.dram_tensor("out", [S, S], BF16, kind="ExternalOutput")

    with tile.TileContext(nc) as tc:
        with (
            tc.tile_pool(name="const", bufs=1) as const,
            tc.tile_pool(name="big", bufs=1) as big,
            tc.tile_pool(name="outp", bufs=4) as outp,
            tc.tile_pool(name="mm_ps", bufs=8, space="PSUM") as mm_ps,
        ):
            warm = const.tile([P, NH], BF16)
            shead_sb = const.tile([P, SO], F32)
            sdep_sb = const.tile([1, S], F32)
            sdep_full = const.tile([P, S], F32)

            u_sb = big.tile([P, DO, DO, P], BF16, tag="u")    # [dd, kt, do, k]
            ht_sb = big.tile([P, DO, S], BF16, tag="ht")      # [dd, dt, i]
            pt_sb = big.tile([P, DO, S], BF16, tag="pt")      # [kk, kt, j]
            t1t_sb = big.tile([P, DO, S], BF16, tag="t1t")    # [kk, kt, i]

            nc.gpsimd.memset(warm[:], 0.0)

            # ---------- DMA dispatch ----------
            # Single sync-ring FIFO = exact priority order, matching mm1's
            # consumption: U column-pairs interleaved with HT slabs, then the
            # epilogue vectors, then PT (first needed by mm2 ~45us in).
            nc.sync.dma_start(u_sb[:, 0:1, :, :], u_d[0][:, 0:1])
            nc.sync.dma_start(ht_sb[:, 0:1, :], ht_d[0][:, 0:1])
            nc.sync.dma_start(ht_sb[:, 1:2, :], ht_d[0][:, 1:2])
            nc.sync.dma_start(u_sb[:, 1:2, :, :], u_d[0][:, 1:2])
            nc.sync.dma_start(ht_sb[:, 2:4, :], ht_d[1])
            nc.sync.dma_start(u_sb[:, 2:4, :, :], u_d[1])
            nc.sync.dma_start(ht_sb[:, 4:6, :], ht_d[2])
            nc.sync.dma_start(u_sb[:, 4:6, :, :], u_d[2])
            nc.sync.dma_start(ht_sb[:, 6:8, :], ht_d[3])
            nc.sync.dma_start(u_sb[:, 6:8, :, :], u_d[3])
            nc.sync.dma_start(shead_sb[:], shead_d[:])
            nc.sync.dma_start(sdep_sb[:], sdep_d[:])
            nc.sync.dma_start(pt_sb[:, 0:4, :], pt_d[0])
            nc.sync.dma_start(pt_sb[:, 4:8, :], pt_d[1])

            # ---------- PE warmup while the first DMAs land ----------
            # distinct lhsT slices per iteration: identical matmuls would be
            # deduplicated by inst_simplify
            warm_ps = mm_ps.tile([P, NH], F32, tag="mm")
            for i in range(N_WARMUP):
                nc.tensor.matmul(warm_ps[:], warm[:, i:i + P], warm[:],
                                 start=True, stop=True)

            # s_dep broadcast row for the epilogue
            for jh in range(2):
                nc.gpsimd.partition_broadcast(
                    sdep_full[:, jh * NH:(jh + 1) * NH],
                    sdep_sb[0:1, jh * NH:(jh + 1) * NH],
                )

            # ---------- mm1: T1T[k, i] ----------
            # Each (kt, ih) accumulation group is emitted in two parts into
            # the SAME psum bank: part A (do 0..3, start=True stop=False)
            # and part B (do 4..7, start=False stop=True) + copy to t1t.
            # Seven part-A chains go first — they only touch ht[0:4]+u, so
            # the PE has ~6us of work queued before it needs the tail of the
            # HT load; B parts and the remaining A parts then interleave,
            # recycling psum banks with ~3 chains of slack.
            def mm1_chain(ps, kt, ih, do_lo, do_hi):
                for do in range(do_lo, do_hi):
                    nc.tensor.matmul(
                        ps[:],
                        u_sb[:, kt, do, :],
                        ht_sb[:, do, ih * NH:(ih + 1) * NH],
                        start=(do == 0),
                        stop=(do == DO - 1),
                    )

            groups = [(kt, ih) for kt in range(DO) for ih in range(2)]
            g_ps = {}
            pend_a = list(groups)
            copy_i = [0]

            def emit_a(g):
                ps = mm_ps.tile([P, NH], F32, tag="mm")
                mm1_chain(ps, g[0], g[1], 0, 4)
                g_ps[g] = ps

            def emit_b(g):
                kt, ih = g
                ps = g_ps.pop(g)
                mm1_chain(ps, kt, ih, 4, DO)
                dst = t1t_sb[:, kt, ih * NH:(ih + 1) * NH]
                if copy_i[0] % 2 == 0:
                    nc.scalar.copy(dst, ps[:])
                else:
                    nc.vector.tensor_copy(dst, ps[:])
                copy_i[0] += 1

            for _ in range(7):          # fill 7 of 8 PSUM banks with A parts
                emit_a(pend_a.pop(0))
            for i, g in enumerate(groups):
                emit_b(g)
                if pend_a:
                    emit_a(pend_a.pop(0))

            # ---------- mm2 + epilogue ----------
            def mm2_group(it, jh, split=1):
                ps = mm_ps.tile([P, NH], F32, tag="mm")
                for kt in range(DO):
                    nc.tensor.matmul(
                        ps[:],
                        t1t_sb[:, kt, it * P:(it + 1) * P],
                        pt_sb[:, kt, jh * NH:(jh + 1) * NH],
                        start=(kt == 0),
                        stop=(kt == DO - 1),
                    )
                ot = outp.tile([P, NH], BF16, tag="out")
                w = NH // split
                for s in range(split):
                    sl = slice(s * w, (s + 1) * w)
                    nc.vector.scalar_tensor_tensor(
                        out=ot[:, sl], in0=ps[:, sl],
                        scalar=shead_sb[:, it:it + 1],
                        in1=sdep_full[:, jh * NH + s * w:jh * NH + (s + 1) * w],
                        op0=ADD, op1=ADD,
                    )
                    nc.sync.dma_start(
                        out_d[it * P:(it + 1) * P,
                              jh * NH + s * w:jh * NH + (s + 1) * w],
                        ot[:, sl],
                    )

            for it in range(SO):
                for jh in range(2):
                    mm2_group(it, jh)

    nc.compile()
    return nc


def _get_nc():
    if "nc" not in _CACHE:
        _CACHE["nc"] = build_nc()
    return _CACHE["nc"]


def _in_maps(head, dep, edge_U, edge_W, edge_b):
    head = np.asarray(head, dtype=np.float32)
    dep = np.asarray(dep, dtype=np.float32)
    edge_U = np.asarray(edge_U, dtype=np.float32)
    w = np.asarray(edge_W, dtype=np.float32).reshape(-1)
    w1, w2 = w[:D], w[D:]
    b0 = float(np.asarray(edge_b, dtype=np.float32).reshape(-1)[0])

    # u[s, dd, kt', do, k] = U[do*P+dd, (2s+kt')*P+k]
    u_prep = np.ascontiguousarray(
        edge_U.reshape(DO, P, DO // 2, 2, P)
        .transpose(2, 1, 3, 0, 4).astype(BF16NP)
    )
    s_head = head @ w1                     # [B, S]
    s_dep = dep @ w2 + b0                  # [B, S]

    maps = []
    for b in range(B):
        # ht[s, dd, j, i] = head[b][i, (2s+j)*P+dd]
        ht = np.ascontiguousarray(
            head[b].T.reshape(DO // 2, 2, P, S).transpose(0, 2, 1, 3)
            .astype(BF16NP)
        )
        # pt[t, kk, m, j] = dep[b][j, (4t+m)*P+kk]
        pt = np.ascontiguousarray(
            dep[b].T.reshape(2, DO // 2, P, S).transpose(0, 2, 1, 3)
            .astype(BF16NP)
        )
        maps.append({
            "ht": ht,
            "pt": pt,
            "u": u_prep,
            "shead": np.ascontiguousarray(s_head[b].reshape(SO, P).T),
            "sdep": np.ascontiguousarray(s_dep[b].reshape(1, S)),
        })
    return maps


def kernel(head, dep, edge_U, edge_W, edge_b, **run_kwargs):
    nc = _get_nc()
    maps = _in_maps(head, dep, edge_U, edge_W, edge_b)
    res = run_bass_kernel_spmd(nc, maps, core_ids=list(range(B)), **run_kwargs)
    out = np.stack(
        [np.asarray(res.results[c]["out"]) for c in range(B)], axis=0
    ).astype(np.float32)
    if run_kwargs:
        _CACHE["last_result"] = res
    return out


# revision 19
# speedup vs baseline: 1.0022x; 1.0022x over previous
"""Biaffine edge attention on 8 Trainium2 NeuronCores (bf16, host relayout).

out[b,i,j] = head[b,i,:] @ edge_U @ dep[b,j,:] + head[b,i,:]@w1 + dep[b,j,:]@w2 + b0

Sharding: data-parallel over batch (B=8, one batch per core). Device does the
two big GEMMs in bf16 (full PE column rate, fp32 PSUM accumulation):
  T1T[k,i] = sum_d U[d,k] * HT[d,i]          (mm1)
  out[i,j] = sum_k T1T[k,i] * PT[k,j] + s_head[i] + s_dep[j] + b0   (mm2 + STT)

Host prep (extends the previous U relayout): HT = head[b].T and PT = dep[b].T
are laid out so every DMA is 128 partitions x large contiguous chunks; the
rank-1 terms s_head = head@w1 and s_dep = dep@w2 + b0 (0.4% of FLOPs) are
computed on host and enter the epilogue as per-partition scalar + broadcast
row. This removes all PE transposes and the sdep matmuls from the PE stream,
leaving exactly the 256 unavoidable 512-column matmuls.

Warmup matmuls on a zeroed tile run while the first DMAs land so the HAM
clock ramp (0.86 -> 1.27 -> 2.4 GHz, activity-driven) completes before the
real stream. All loads ride the sync HWDGE ring in consumption order (the
ACT ring takes ~5us to start transferring); the first U column-pair + HT
slab dispatches are small so mm1 can start after ~0.5MB instead of 6MB --
with 8 cores loading at once the per-core HBM share is only ~200GB/s.
"""

import numpy as np
import ml_dtypes

import concourse.bass as bass  # noqa: F401  (side-effect: mybir registration)
import concourse.mybir as mybir
import concourse.tile as tile
from concourse import bacc
from concourse.bass_utils import run_bass_kernel_spmd

B, S, D = 8, 1024, 1024
P = 128
SO = S // P  # 8
DO = D // P  # 8
NH = 512     # matmul free-dim tile (one fp32 PSUM bank)
F32 = mybir.dt.float32
BF16 = mybir.dt.bfloat16
ADD = mybir.AluOpType.add
BF16NP = ml_dtypes.bfloat16

N_WARMUP = 12

_CACHE = {}


def build_nc():
    nc = bacc.Bacc(None, target_bir_lowering=False)

    # Slab-major host layouts; each dispatch reads one dense block:
    # ht[s, dd, j, i] = headT[(2s+j)*P + dd, i].
    ht_d = nc.dram_tensor("ht", [DO // 2, P, 2, S], BF16, kind="ExternalInput")
    pt_d = nc.dram_tensor("pt", [2, P, DO // 2, S], BF16, kind="ExternalInput")
    # u[s, dd, kt', do, k] = U[do*P+dd, (2s+kt')*P+k]
    u_d = nc.dram_tensor("u", [DO // 2, P, 2, DO, P], BF16,
                         kind="ExternalInput")
    shead_d = nc.dram_tensor("shead", [P, SO], F32, kind="ExternalInput")
    sdep_d = nc.dram_tensor("sdep", [1, S], F32, kind="ExternalInput")
    out_d = nc.dram_tensor("out", [S, S], BF16, kind="ExternalOutput")

    with tile.TileContext(nc) as tc:
        with (
            tc.tile_pool(name="const", bufs=1) as const,
            tc.tile_pool(name="big", bufs=1) as big,
            tc.tile_pool(name="outp", bufs=4) as outp,
            tc.tile_pool(name="mm_ps", bufs=8, space="PSUM") as mm_ps,
        ):
            warm = const.tile([P, NH], BF16)
            shead_sb = const.tile([P, SO], F32)
            sdep_sb = const.tile([1, S], F32)
            sdep_full = const.tile([P, S], F32)

            u_sb = big.tile([P, DO, DO, P], BF16, tag="u")    # [dd, kt, do, k]
            ht_sb = big.tile([P, DO, S], BF16, tag="ht")      # [dd, dt, i]
            pt_sb = big.tile([P, DO, S], BF16, tag="pt")      # [kk, kt, j]
            t1t_sb = big.tile([P, DO, S], BF16, tag="t1t")    # [kk, kt, i]

            nc.gpsimd.memset(warm[:], 0.0)

            # ---------- DMA dispatch ----------
            # Single sync-ring FIFO = exact priority order, matching mm1's
            # consumption: U column-pairs interleaved with HT slabs, then the
            # epilogue vectors, then PT (first needed by mm2 ~45us in).
            nc.sync.dma_start(u_sb[:, 0:1, :, :], u_d[0][:, 0:1])
            nc.sync.dma_start(ht_sb[:, 0:1, :], ht_d[0][:, 0:1])
            nc.sync.dma_start(ht_sb[:, 1:2, :], ht_d[0][:, 1:2])
            nc.sync.dma_start(u_sb[:, 1:2, :, :], u_d[0][:, 1:2])
            nc.sync.dma_start(ht_sb[:, 2:4, :], ht_d[1])
            nc.sync.dma_start(u_sb[:, 2:4, :, :], u_d[1])
            nc.sync.dma_start(ht_sb[:, 4:6, :], ht_d[2])
            nc.sync.dma_start(u_sb[:, 4:6, :, :], u_d[2])
            nc.sync.dma_start(ht_sb[:, 6:8, :], ht_d[3])
            nc.sync.dma_start(u_sb[:, 6:8, :, :], u_d[3])
            nc.sync.dma_start(shead_sb[:], shead_d[:])
            nc.sync.dma_start(sdep_sb[:], sdep_d[:])
            nc.sync.dma_start(pt_sb[:, 0:4, :], pt_d[0])
            nc.sync.dma_start(pt_sb[:, 4:8, :], pt_d[1])

            # ---------- PE warmup while the first DMAs land ----------
            # distinct lhsT slices per iteration: identical matmuls would be
            # deduplicated by inst_simplify
            warm_ps = mm_ps.tile([P, NH], F32, tag="mm")
            for i in range(N_WARMUP):
                nc.tensor.matmul(warm_ps[:], warm[:, i:i + P], warm[:],
                                 start=True, stop=True)

            # s_dep broadcast row for the epilogue
            for jh in range(2):
                nc.gpsimd.partition_broadcast(
                    sdep_full[:, jh * NH:(jh + 1) * NH],
                    sdep_sb[0:1, jh * NH:(jh + 1) * NH],
                )

            # ---------- mm1: T1T[k, i] ----------
            # Each (kt, ih) accumulation group is emitted in two parts into
            # the SAME psum bank: part A (do 0..3, start=True stop=False)
            # and part B (do 4..7, start=False stop=True) + copy to t1t.
            # Seven part-A chains go first — they only touch ht[0:4]+u, so
            # the PE has ~6us of work queued before it needs the tail of the
            # HT load; B parts and the remaining A parts then interleave,
            # recycling psum banks with ~3 chains of slack.
            def mm1_chain(ps, kt, ih, do_lo, do_hi):
                for do in range(do_lo, do_hi):
                    nc.tensor.matmul(
                        ps[:],
                        u_sb[:, kt, do, :],
                        ht_sb[:, do, ih * NH:(ih + 1) * NH],
                        start=(do == 0),
                        stop=(do == DO - 1),
                    )

            groups = [(kt, ih) for kt in range(DO) for ih in range(2)]
            g_ps = {}
            pend_a = list(groups)
            copy_i = [0]

            def emit_a(g):
                ps = mm_ps.tile([P, NH], F32, tag="mm")
                mm1_chain(ps, g[0], g[1], 0, 4)
                g_ps[g] = ps

            def emit_b(g):
                kt, ih = g
                ps = g_ps.pop(g)
                mm1_chain(ps, kt, ih, 4, DO)
                dst = t1t_sb[:, kt, ih * NH:(ih + 1) * NH]
                if copy_i[0] % 2 == 0:
                    nc.scalar.copy(dst, ps[:])
                else:
                    nc.vector.tensor_copy(dst, ps[:])
                copy_i[0] += 1

            for _ in range(7):          # fill 7 of 8 PSUM banks with A parts
                emit_a(pend_a.pop(0))
            for i, g in enumerate(groups):
                emit_b(g)
                if pend_a:
                    emit_a(pend_a.pop(0))

            # ---------- mm2 + epilogue ----------
            # split=2 on the final group: two 256-col accumulation chains in
            # one bank, each followed immediately by its STT + out dispatch,
            # so the end-of-kernel latency chain is half a tile long.
            def mm2_group(it, jh, split=1):
                ps = mm_ps.tile([P, NH], F32, tag="mm")
                ot = outp.tile([P, NH], BF16, tag="out")
                wq = NH // split
                for s in range(split):
                    sl = slice(s * wq, (s + 1) * wq)
                    for kt in range(DO):
                        nc.tensor.matmul(
                            ps[:, sl],
                            t1t_sb[:, kt, it * P:(it + 1) * P],
                            pt_sb[:, kt,
                                  jh * NH + s * wq:jh * NH + (s + 1) * wq],
                            start=(kt == 0),
                            stop=(kt == DO - 1),
                        )
                    nc.vector.scalar_tensor_tensor(
                        out=ot[:, sl], in0=ps[:, sl],
                        scalar=shead_sb[:, it:it + 1],
                        in1=sdep_full[:, jh * NH + s * wq:jh * NH + (s + 1) * wq],
                        op0=ADD, op1=ADD,
                    )
                    nc.sync.dma_start(
                        out_d[it * P:(it + 1) * P,
                              jh * NH + s * wq:jh * NH + (s + 1) * wq],
                        ot[:, sl],
                    )

            for it in range(SO):
                for jh in range(2):
                    mm2_group(it, jh,
                              split=(2 if (it, jh) == (SO - 1, 1) else 1))

    nc.compile()
    return nc


def _get_nc():
    if "nc" not in _CACHE:
        _CACHE["nc"] = build_nc()
    return _CACHE["nc"]


def _in_maps(head, dep, edge_U, edge_W, edge_b):
    head = np.asarray(head, dtype=np.float32)
    dep = np.asarray(dep, dtype=np.float32)
    edge_U = np.asarray(edge_U, dtype=np.float32)
    w = np.asarray(edge_W, dtype=np.float32).reshape(-1)
    w1, w2 = w[:D], w[D:]
    b0 = float(np.asarray(edge_b, dtype=np.float32).reshape(-1)[0])

    # u[s, dd, kt', do, k] = U[do*P+dd, (2s+kt')*P+k]
    u_prep = np.ascontiguousarray(
        edge_U.reshape(DO, P, DO // 2, 2, P)
        .transpose(2, 1, 3, 0, 4).astype(BF16NP)
    )
    s_head = head @ w1                     # [B, S]
    s_dep = dep @ w2 + b0                  # [B, S]

    maps = []
    for b in range(B):
        # ht[s, dd, j, i] = head[b][i, (2s+j)*P+dd]
        ht = np.ascontiguousarray(
            head[b].T.reshape(DO // 2, 2, P, S).transpose(0, 2, 1, 3)
            .astype(BF16NP)
        )
        # pt[t, kk, m, j] = dep[b][j, (4t+m)*P+kk]
        pt = np.ascontiguousarray(
            dep[b].T.reshape(2, DO // 2, P, S).transpose(0, 2, 1, 3)
            .astype(BF16NP)
        )
        maps.append({
            "ht": ht,
            "pt": pt,
            "u": u_prep,
            "shead": np.ascontiguousarray(s_head[b].reshape(SO, P).T),
            "sdep": np.ascontiguousarray(s_dep[b].reshape(1, S)),
        })
    return maps


def kernel(head, dep, edge_U, edge_W, edge_b, **run_kwargs):
    nc = _get_nc()
    maps = _in_maps(head, dep, edge_U, edge_W, edge_b)
    res = run_bass_kernel_spmd(nc, maps, core_ids=list(range(B)), **run_kwargs)
    out = np.stack(
        [np.asarray(res.results[c]["out"]) for c in range(B)], axis=0
    ).astype(np.float32)
    if run_kwargs:
        _CACHE["last_result"] = res
    return out


# revision 21
# speedup vs baseline: 1.0205x; 1.0182x over previous
"""Biaffine edge attention on 8 Trainium2 NeuronCores (bf16, host relayout).

out[b,i,j] = head[b,i,:] @ edge_U @ dep[b,j,:] + head[b,i,:]@w1 + dep[b,j,:]@w2 + b0

Sharding: data-parallel over batch (B=8, one batch per core). Device does the
two big GEMMs in bf16 (full PE column rate, fp32 PSUM accumulation):
  T1T[k,i] = sum_d U[d,k] * HT[d,i]          (mm1)
  out[i,j] = sum_k T1T[k,i] * PT[k,j] + s_head[i] + s_dep[j] + b0   (mm2 + STT)

Host prep (extends the previous U relayout): HT = head[b].T and PT = dep[b].T
are laid out so every DMA is 128 partitions x large contiguous chunks; the
rank-1 terms s_head = head@w1 and s_dep = dep@w2 + b0 (0.4% of FLOPs) are
computed on host and enter the epilogue as per-partition scalar + broadcast
row. This removes all PE transposes and the sdep matmuls from the PE stream,
leaving exactly the 256 unavoidable 512-column matmuls.

Warmup matmuls on a zeroed tile run while the first DMAs land so the HAM
clock ramp (0.86 -> 1.27 -> 2.4 GHz, activity-driven) completes before the
real stream. All loads ride the sync HWDGE ring in consumption order (the
ACT ring takes ~5us to start transferring); the first U column-pair + HT
slab dispatches are small so mm1 can start after ~0.5MB instead of 6MB --
with 8 cores loading at once the per-core HBM share is only ~200GB/s.
"""

import numpy as np
import ml_dtypes

import concourse.bass as bass  # noqa: F401  (side-effect: mybir registration)
import concourse.mybir as mybir
import concourse.tile as tile
from concourse import bacc
from concourse.bass_utils import run_bass_kernel_spmd

B, S, D = 8, 1024, 1024
P = 128
SO = S // P  # 8
DO = D // P  # 8
NH = 512     # matmul free-dim tile (one fp32 PSUM bank)
F32 = mybir.dt.float32
BF16 = mybir.dt.bfloat16
ADD = mybir.AluOpType.add
BF16NP = ml_dtypes.bfloat16

N_WARMUP = 10

_CACHE = {}


def build_nc():
    nc = bacc.Bacc(None, target_bir_lowering=False)

    # Slab-major host layouts; each dispatch reads one dense block:
    # ht[s, dd, j, i] = headT[(2s+j)*P + dd, i].
    ht_d = nc.dram_tensor("ht", [DO // 2, P, 2, S], BF16, kind="ExternalInput")
    pt_d = nc.dram_tensor("pt", [2, P, DO // 2, S], BF16, kind="ExternalInput")
    # u[s, dd, kt', do, k] = U[do*P+dd, (2s+kt')*P+k]
    u_d = nc.dram_tensor("u", [DO // 2, P, 2, DO, P], BF16,
                         kind="ExternalInput")
    shead_d = nc.dram_tensor("shead", [P, SO], F32, kind="ExternalInput")
    sdep_d = nc.dram_tensor("sdep", [1, S], F32, kind="ExternalInput")
    out_d = nc.dram_tensor("out", [S, S], BF16, kind="ExternalOutput")

    with tile.TileContext(nc) as tc:
        with (
            tc.tile_pool(name="const", bufs=1) as const,
            tc.tile_pool(name="big", bufs=1) as big,
            tc.tile_pool(name="outp", bufs=4) as outp,
            tc.tile_pool(name="mm_ps", bufs=8, space="PSUM") as mm_ps,
        ):
            warm = const.tile([P, NH], BF16)
            shead_sb = const.tile([P, SO], F32)
            sdep_sb = const.tile([1, S], F32)
            sdep_full = const.tile([P, S], F32)

            u_sb = big.tile([P, DO, DO, P], BF16, tag="u")    # [dd, kt, do, k]
            ht_sb = big.tile([P, DO, S], BF16, tag="ht")      # [dd, dt, i]
            pt_sb = big.tile([P, DO, S], BF16, tag="pt")      # [kk, kt, j]
            t1t_sb = big.tile([P, DO, S], BF16, tag="t1t")    # [kk, kt, i]

            nc.gpsimd.memset(warm[:], 0.0)

            # ---------- DMA dispatch ----------
            # Single sync-ring FIFO = exact priority order, matching mm1's
            # consumption: U column-pairs interleaved with HT slabs, then the
            # epilogue vectors, then PT (first needed by mm2 ~45us in).
            nc.sync.dma_start(u_sb[:, 0:1, :, :], u_d[0][:, 0:1])
            nc.sync.dma_start(ht_sb[:, 0:1, :], ht_d[0][:, 0:1])
            nc.sync.dma_start(ht_sb[:, 1:2, :], ht_d[0][:, 1:2])
            nc.sync.dma_start(u_sb[:, 1:2, :, :], u_d[0][:, 1:2])
            nc.sync.dma_start(ht_sb[:, 2:4, :], ht_d[1])
            nc.sync.dma_start(u_sb[:, 2:4, :, :], u_d[1])
            nc.sync.dma_start(ht_sb[:, 4:6, :], ht_d[2])
            nc.sync.dma_start(u_sb[:, 4:6, :, :], u_d[2])
            nc.sync.dma_start(ht_sb[:, 6:8, :], ht_d[3])
            nc.sync.dma_start(u_sb[:, 6:8, :, :], u_d[3])
            nc.sync.dma_start(shead_sb[:], shead_d[:])
            nc.sync.dma_start(sdep_sb[:], sdep_d[:])
            nc.sync.dma_start(pt_sb[:, 0:4, :], pt_d[0])
            nc.sync.dma_start(pt_sb[:, 4:8, :], pt_d[1])

            # ---------- PE warmup while the first DMAs land ----------
            # distinct lhsT slices per iteration: identical matmuls would be
            # deduplicated by inst_simplify
            warm_ps = mm_ps.tile([P, NH], F32, tag="mm")
            for i in range(N_WARMUP):
                nc.tensor.matmul(warm_ps[:], warm[:, i:i + P], warm[:],
                                 start=True, stop=True)

            # s_dep broadcast row for the epilogue
            for jh in range(2):
                nc.gpsimd.partition_broadcast(
                    sdep_full[:, jh * NH:(jh + 1) * NH],
                    sdep_sb[0:1, jh * NH:(jh + 1) * NH],
                )

            # ---------- mm1: T1T[k, i] ----------
            # Each (kt, ih) accumulation group is emitted in two parts into
            # the SAME psum bank: part A (do 0..3, start=True stop=False)
            # and part B (do 4..7, start=False stop=True) + copy to t1t.
            # Seven part-A chains go first — they only touch ht[0:4]+u, so
            # the PE has ~6us of work queued before it needs the tail of the
            # HT load; B parts and the remaining A parts then interleave,
            # recycling psum banks with ~3 chains of slack.
            def mm1_chain(ps, kt, ih, do_lo, do_hi):
                for do in range(do_lo, do_hi):
                    nc.tensor.matmul(
                        ps[:],
                        u_sb[:, kt, do, :],
                        ht_sb[:, do, ih * NH:(ih + 1) * NH],
                        start=(do == 0),
                        stop=(do == DO - 1),
                    )

            groups = [(kt, ih) for kt in range(DO) for ih in range(2)]
            g_ps = {}
            pend_a = list(groups)
            copy_i = [0]

            def emit_a(g):
                ps = mm_ps.tile([P, NH], F32, tag="mm")
                mm1_chain(ps, g[0], g[1], 0, 4)
                g_ps[g] = ps

            def emit_b(g):
                kt, ih = g
                ps = g_ps.pop(g)
                mm1_chain(ps, kt, ih, 4, DO)
                dst = t1t_sb[:, kt, ih * NH:(ih + 1) * NH]
                if copy_i[0] % 2 == 0:
                    nc.scalar.copy(dst, ps[:])
                else:
                    nc.vector.tensor_copy(dst, ps[:])
                copy_i[0] += 1

            for _ in range(7):          # fill 7 of 8 PSUM banks with A parts
                emit_a(pend_a.pop(0))
            for i, g in enumerate(groups):
                emit_b(g)
                if pend_a:
                    emit_a(pend_a.pop(0))

            # ---------- mm2 + epilogue ----------
            # split=2 on the final group: two 256-col accumulation chains in
            # one bank, each followed immediately by its STT + out dispatch,
            # so the end-of-kernel latency chain is half a tile long.
            def mm2_group(it, jh, split=1):
                ot = outp.tile([P, NH], BF16, tag="out")
                wq = NH // split
                for s in range(split):
                    # fresh psum tile per sub-chain: half 2's matmuls must not
                    # WAR-serialize against half 1's STT read
                    ps = mm_ps.tile([P, NH], F32, tag="mm")
                    sl = slice(s * wq, (s + 1) * wq)
                    for kt in range(DO):
                        nc.tensor.matmul(
                            ps[:, sl],
                            t1t_sb[:, kt, it * P:(it + 1) * P],
                            pt_sb[:, kt,
                                  jh * NH + s * wq:jh * NH + (s + 1) * wq],
                            start=(kt == 0),
                            stop=(kt == DO - 1),
                        )
                    nc.vector.scalar_tensor_tensor(
                        out=ot[:, sl], in0=ps[:, sl],
                        scalar=shead_sb[:, it:it + 1],
                        in1=sdep_full[:, jh * NH + s * wq:jh * NH + (s + 1) * wq],
                        op0=ADD, op1=ADD,
                    )
                    nc.sync.dma_start(
                        out_d[it * P:(it + 1) * P,
                              jh * NH + s * wq:jh * NH + (s + 1) * wq],
                        ot[:, sl],
                    )

            for it in range(SO):
                for jh in range(2):
                    mm2_group(it, jh,
                              split=(2 if (it, jh) == (SO - 1, 1) else 1))

    nc.compile()
    return nc


def _get_nc():
    if "nc" not in _CACHE:
        _CACHE["nc"] = build_nc()
    return _CACHE["nc"]


def _in_maps(head, dep, edge_U, edge_W, edge_b):
    head = np.asarray(head, dtype=np.float32)
    dep = np.asarray(dep, dtype=np.float32)
    edge_U = np.asarray(edge_U, dtype=np.float32)
    w = np.asarray(edge_W, dtype=np.float32).reshape(-1)
    w1, w2 = w[:D], w[D:]
    b0 = float(np.asarray(edge_b, dtype=np.float32).reshape(-1)[0])

    # u[s, dd, kt', do, k] = U[do*P+dd, (2s+kt')*P+k]
    u_prep = np.ascontiguousarray(
        edge_U.reshape(DO, P, DO // 2, 2, P)
        .transpose(2, 1, 3, 0, 4).astype(BF16NP)
    )
    s_head = head @ w1                     # [B, S]
    s_dep = dep @ w2 + b0                  # [B, S]

    maps = []
    for b in range(B):
        # ht[s, dd, j, i] = head[b][i, (2s+j)*P+dd]
        ht = np.ascontiguousarray(
            head[b].T.reshape(DO // 2, 2, P, S).transpose(0, 2, 1, 3)
            .astype(BF16NP)
        )
        # pt[t, kk, m, j] = dep[b][j, (4t+m)*P+kk]
        pt = np.ascontiguousarray(
            dep[b].T.reshape(2, DO // 2, P, S).transpose(0, 2, 1, 3)
            .astype(BF16NP)
        )
        maps.append({
            "ht": ht,
            "pt": pt,
            "u": u_prep,
            "shead": np.ascontiguousarray(s_head[b].reshape(SO, P).T),
            "sdep": np.ascontiguousarray(s_dep[b].reshape(1, S)),
        })
    return maps


def kernel(head, dep, edge_U, edge_W, edge_b, **run_kwargs):
    nc = _get_nc()
    maps = _in_maps(head, dep, edge_U, edge_W, edge_b)
    res = run_bass_kernel_spmd(nc, maps, core_ids=list(range(B)), **run_kwargs)
    out = np.stack(
        [np.asarray(res.results[c]["out"]) for c in range(B)], axis=0
    ).astype(np.float32)
    if run_kwargs:
        _CACHE["last_result"] = res
    return out
